# revision 8
# baseline (speedup 1.0000x reference)
"""AttentionBlock3D (GroupNorm + 8-head self-attention over 16^3 voxels +
out-projection + residual) on 8 TRN2 NeuronCores.

Sharding: one attention head per core (H=8). Every core:
  - loads the full x [64, 4096], computes GroupNorm (stats via bn_stats +
    block-diag matmul group-combine),
  - computes its head's q,k (one [64,16] matmul) and v^T (per-t-block
    matmuls producing the transposed v directly),
  - streams flash-attention-style over the 4096x4096 score matrix:
    scores^T tile = k_blk^T q  (PE, fp16), exp on ScalarE (PSUM->SBUF,
    fused *scale, -6.0 offset; constant offset cancels in softmax),
    out_aug accumulation via PE with v^T augmented by a ones column so the
    softmax denominator falls out of the same matmul,
  - divides, projects through its slice of out_w, writes a partial [64,4096].
Host gathers: out = sum(partials) + out_b + x.
"""
import os
from contextlib import ExitStack

import numpy as np

import concourse.bass as bass
import concourse.tile as tile
from concourse import bacc, mybir
from concourse.bass import ts
from concourse.bass_utils import run_bass_kernel_spmd

C, H, G, D = 64, 8, 8, 8
S = 4096
EPS = 1e-5
SCALE = float(D) ** -0.5
EXP_OFF = -6.0          # constant exp offset; cancels in softmax division

SC = 512                # s-chunk (one PSUM bank of fp32)
NSC = S // SC           # 8
TB = 128                # t-block (partition dim of scores^T tiles)
NTB = S // TB           # 32
BT = 3                  # t-blocks per exp batch (3 PSUM banks per scores tile)

F32 = mybir.dt.float32
F16 = mybir.dt.float16

_NC_CACHE = None


def _emit(nc, reps=1):
    x = nc.dram_tensor("x", [C, S], F32, kind="ExternalInput").ap()
    gamma = nc.dram_tensor("gamma", [C, 1], F32, kind="ExternalInput").ap()
    beta = nc.dram_tensor("beta", [C, 1], F32, kind="ExternalInput").ap()
    gdiag = nc.dram_tensor("gdiag", [C, C], F32, kind="ExternalInput").ap()
    wqk = nc.dram_tensor("wqk", [C, 2 * D], F32, kind="ExternalInput").ap()
    wv = nc.dram_tensor("wv", [C, D], F32, kind="ExternalInput").ap()
    wo = nc.dram_tensor("wo", [D, C], F32, kind="ExternalInput").ap()
    part = nc.dram_tensor("part", [C, S], F32, kind="ExternalOutput").ap()

    with tile.TileContext(nc) as tc:
        if reps == 1:
            _body(nc, tc, x, gamma, beta, gdiag, wqk, wv, wo, part)
        else:
            # benchmark variant: repeat the whole kernel body on-device
            with tc.For_i(0, reps, 1, hint_engines=(mybir.EngineType.PE,)):
                _body(nc, tc, x, gamma, beta, gdiag, wqk, wv, wo, part)


def _body(nc, tc, x, gamma, beta, gdiag, wqk, wv, wo, part):
    with ExitStack() as ctx:
        const = ctx.enter_context(tc.tile_pool(name="const", bufs=1))
        big = ctx.enter_context(tc.tile_pool(name="big", bufs=1))
        small = ctx.enter_context(tc.tile_pool(name="small", bufs=1))

        # ---- load inputs ----
        x_sb = big.tile([C, S], F32, name="x_sb")
        nc.sync.dma_start(out=x_sb[:], in_=x)
        gamma_sb = const.tile([C, 1], F32, name="gamma_sb")
        nc.sync.dma_start(out=gamma_sb[:], in_=gamma)
        beta_sb = const.tile([C, 1], F32, name="beta_sb")
        nc.sync.dma_start(out=beta_sb[:], in_=beta)
        gdiag_sb = const.tile([C, C], F32, name="gdiag_sb")
        nc.sync.dma_start(out=gdiag_sb[:], in_=gdiag)
        wqk_sb = const.tile([C, 2 * D], F32, name="wqk_sb")
        nc.sync.dma_start(out=wqk_sb[:], in_=wqk)
        wv_sb = const.tile([C, D], F32, name="wv_sb")
        nc.sync.dma_start(out=wv_sb[:], in_=wv)
        wo_sb = const.tile([D, C], F32, name="wo_sb")
        nc.sync.dma_start(out=wo_sb[:], in_=wo)
        eps_sb = const.tile([C, 1], F32, name="eps_sb")
        nc.vector.memset(eps_sb[:], EPS)
        ones_sb = const.tile([1, D], F32, name="ones_sb")
        nc.vector.memset(ones_sb[:], 1.0)
        zero_sb = const.tile([C, 1], F32, name="zero_sb")
        nc.vector.memset(zero_sb[:], 0.0)
        expoff_sb = const.tile([TB, 1], F32, name="expoff_sb")
        nc.vector.memset(expoff_sb[:], EXP_OFF)

        # ---- GroupNorm stats (per-channel bn_stats, then group combine) ----
        NSUB = S // 512
        stats = small.tile([C, NSUB, 6], F32, name="stats")
        xv = x_sb[:].rearrange("p (n f) -> p n f", f=512)
        for i in range(NSUB):
            nc.vector.bn_stats(out=stats[:, i, :], in_=xv[:, i, :])
        mv = small.tile([C, 2], F32, name="mv")
        nc.vector.bn_aggr(out=mv[:], in_=stats[:])

        # m2 = [mean_c, E[x^2]_c]
        m2 = small.tile([C, 2], F32, name="m2")
        nc.vector.tensor_copy(out=m2[:, 0:1], in_=mv[:, 0:1])
        nc.vector.tensor_mul(out=m2[:, 1:2], in0=mv[:, 0:1], in1=mv[:, 0:1])
        nc.vector.tensor_add(out=m2[:, 1:2], in0=m2[:, 1:2], in1=mv[:, 1:2])

        gst = small.tile([C, 2], F32, name="gst")
        with tc.tile_pool(name="pre_ps", bufs=1, space="PSUM") as pre_ps:
            gst_ps = pre_ps.tile([C, 2], F32, name="gst_ps")
            nc.tensor.matmul(gst_ps[:], lhsT=gdiag_sb[:], rhs=m2[:],
                             start=True, stop=True)
            nc.vector.tensor_copy(out=gst[:], in_=gst_ps[:])

        # var_g = E[x^2]_g - mean_g^2 ; rstd = exp(-0.5*ln(var+eps))
        var = small.tile([C, 1], F32, name="var")
        nc.vector.tensor_mul(out=var[:], in0=gst[:, 0:1], in1=gst[:, 0:1])
        nc.vector.tensor_sub(out=var[:], in0=gst[:, 1:2], in1=var[:])
        rstd = small.tile([C, 1], F32, name="rstd")
        nc.scalar.activation(out=rstd[:], in_=var[:],
                             func=mybir.ActivationFunctionType.Ln,
                             bias=eps_sb[:], scale=1.0)
        nc.scalar.activation(out=rstd[:], in_=rstd[:],
                             func=mybir.ActivationFunctionType.Exp,
                             bias=zero_sb[:], scale=-0.5)
        a_sc = small.tile([C, 1], F32, name="a_sc")
        nc.vector.tensor_mul(out=a_sc[:], in0=rstd[:], in1=gamma_sb[:])
        b_sc = small.tile([C, 1], F32, name="b_sc")
        nc.vector.tensor_mul(out=b_sc[:], in0=gst[:, 0:1], in1=a_sc[:])
        nc.vector.tensor_sub(out=b_sc[:], in0=beta_sb[:], in1=b_sc[:])

        xn_sb = big.tile([C, S], F32, name="xn_sb")
        nc.vector.tensor_scalar(out=xn_sb[:], in0=x_sb[:],
                                scalar1=a_sc[:], scalar2=b_sc[:],
                                op0=mybir.AluOpType.mult,
                                op1=mybir.AluOpType.add)

        # ---- q, k for this head (fp16), one [64,16]x[64,S] matmul ----
        # Engine accesses must start at 32-aligned partitions, so copy the
        # [16,S] PSUM result as one block, then peel k off with a DMA
        # (DMAs may start at any partition).
        qk_sb = big.tile([2 * D, S], F16, name="qk_sb")
        k_sb = big.tile([D, S], F16, name="k_sb")
        with tc.tile_pool(name="qkv_ps", bufs=1, space="PSUM") as qkv_pool:
            qk_ps = qkv_pool.tile([2 * D, S], F32, name="qk_ps")
            for j in range(NSC):
                nc.tensor.matmul(qk_ps[:, ts(j, SC)], lhsT=wqk_sb[:],
                                 rhs=xn_sb[:, ts(j, SC)], start=True, stop=True)
            nc.scalar.copy(out=qk_sb[:], in_=qk_ps[:])
        nc.sync.dma_start(out=k_sb[:], in_=qk_sb[D:2 * D, :])
        q_sb = qk_sb  # rows 0:D are q (base partition 0)

        # ---- v^T padded to 33 cols: 0:8 = v, 8:32 = 0, 32 = ones ----
        # (the PV matmul then emits the softmax denominator on PSUM
        # partition 32, which is a legal engine-access base)
        MAUG = 33
        vT_sb = big.tile([TB, NTB, MAUG], F16, name="vT_sb")
        nc.vector.memset(vT_sb[:], 0.0)
        nc.vector.memset(vT_sb[:, :, MAUG - 1:MAUG], 1.0)
        with tc.tile_pool(name="vt_ps", bufs=1, space="PSUM") as vt_pool:
            vt_ps = vt_pool.tile([TB, NTB, D], F32, name="vt_ps")
            for i in range(NTB):
                nc.tensor.matmul(vt_ps[:, i, :], lhsT=xn_sb[:, ts(i, TB)],
                                 rhs=wv_sb[:], start=True, stop=True)
            nc.scalar.copy(out=vT_sb[:, :, 0:D], in_=vt_ps[:])

        # ---- attention main loop ----
        sc_pool = ctx.enter_context(tc.tile_pool(name="sc_ps", bufs=2, space="PSUM"))
        exp_pool = ctx.enter_context(tc.tile_pool(name="exp_sb", bufs=3))
        outp_pool = ctx.enter_context(tc.tile_pool(name="out_ps", bufs=1, space="PSUM"))
        fin_ps_pool = ctx.enter_context(tc.tile_pool(name="fin_ps", bufs=1, space="PSUM"))
        fin_sb_pool = ctx.enter_context(tc.tile_pool(name="fin_sb", bufs=2))
        osb_pool = ctx.enter_context(tc.tile_pool(name="o_sb", bufs=2))

        batches = [BT] * (NTB // BT) + ([NTB % BT] if NTB % BT else [])

        for s in range(NSC):
            out_ps = outp_pool.tile([MAUG, SC], F32, name="out_ps")
            tb0 = 0
            for nb in batches:
                scp = sc_pool.tile([TB, BT * SC], F32, name="scp")
                expt = exp_pool.tile([TB, BT * SC], F16, name="expt")
                for j in range(nb):
                    t = tb0 + j
                    nc.tensor.matmul(scp[:, ts(j, SC)],
                                     lhsT=k_sb[:, ts(t, TB)],
                                     rhs=q_sb[0:D, ts(s, SC)],
                                     start=True, stop=True)
                nc.scalar.activation(out=expt[:, 0:nb * SC],
                                     in_=scp[:, 0:nb * SC],
                                     func=mybir.ActivationFunctionType.Exp,
                                     bias=expoff_sb[:], scale=SCALE)
                for j in range(nb):
                    t = tb0 + j
                    nc.tensor.matmul(out_ps[:], lhsT=vT_sb[:, t, :],
                                     rhs=expt[:, ts(j, SC)],
                                     start=(t == 0), stop=(t == NTB - 1))
                tb0 += nb

            # finalize: divide by row-sum, project, store
            recip = fin_sb_pool.tile([1, SC], F32, name="recip")
            nc.vector.reciprocal(out=recip[:], in_=out_ps[MAUG - 1:MAUG, :])
            bcast_ps = fin_ps_pool.tile([D, SC], F32, name="bcast_ps", tag="fin")
            nc.tensor.matmul(bcast_ps[:], lhsT=ones_sb[:], rhs=recip[:],
                             start=True, stop=True)
            bcast_sb = fin_sb_pool.tile([D, SC], F32, name="bcast_sb")
            nc.vector.tensor_copy(out=bcast_sb[:], in_=bcast_ps[:])
            attn_sb = fin_sb_pool.tile([D, SC], F32, name="attn_sb")
            nc.vector.tensor_mul(out=attn_sb[:], in0=out_ps[0:D, :],
                                 in1=bcast_sb[:])
            proj_ps = fin_ps_pool.tile([C, SC], F32, name="proj_ps", tag="fin")
            nc.tensor.matmul(proj_ps[:], lhsT=wo_sb[:], rhs=attn_sb[:],
                             start=True, stop=True)
            o_sb = osb_pool.tile([C, SC], F32, name="o_sb")
            nc.vector.tensor_copy(out=o_sb[:], in_=proj_ps[:])
            nc.sync.dma_start(out=part[:, ts(s, SC)], in_=o_sb[:])


_NC_CACHE_REPS = {}


def _build(reps=1):
    global _NC_CACHE_REPS
    if reps in _NC_CACHE_REPS:
        return _NC_CACHE_REPS[reps]
    nc = bacc.Bacc("TRN2", target_bir_lowering=False, debug=False)
    _emit(nc, reps=reps)
    nc.compile()
    _NC_CACHE_REPS[reps] = nc
    return nc


def _host_inputs(inputs):
    x = np.ascontiguousarray(np.asarray(inputs["x"], dtype=np.float32))
    gn_w = np.asarray(inputs["gn_weight"], dtype=np.float32).reshape(C, 1)
    gn_b = np.asarray(inputs["gn_bias"], dtype=np.float32).reshape(C, 1)
    qkv_w = np.asarray(inputs["qkv_w"], dtype=np.float32)
    out_w = np.asarray(inputs["out_w"], dtype=np.float32)

    x2 = np.ascontiguousarray(x.reshape(C, S))
    gd = np.kron(np.eye(G, dtype=np.float32),
                 np.full((C // G, C // G), float(G) / C, dtype=np.float32))
    gd = np.ascontiguousarray(gd)

    in_maps = []
    for h in range(H):
        rq = np.arange(h * D, (h + 1) * D)
        wqk_h = np.ascontiguousarray(
            qkv_w[np.concatenate([rq, C + rq])].T)          # [64, 16]
        wv_h = np.ascontiguousarray(qkv_w[2 * C + rq].T)    # [64, 8]
        wo_h = np.ascontiguousarray(out_w[:, rq].T)         # [8, 64]
        in_maps.append({
            "x": x2, "gamma": gn_w, "beta": gn_b, "gdiag": gd,
            "wqk": wqk_h, "wv": wv_h, "wo": wo_h,
        })
    return in_maps, x2


def kernel(**inputs):
    x = np.asarray(inputs["x"])
    out_b = np.asarray(inputs["out_b"], dtype=np.float32)
    in_maps, x2 = _host_inputs(inputs)

    nc = _build()
    trace = bool(int(os.environ.get("KERNEL_TRACE", "0")))
    res = run_bass_kernel_spmd(nc, in_maps, core_ids=list(range(H)),
                               trace=trace)
    if trace:
        kernel.last_results = res

    acc = np.zeros((C, S), dtype=np.float32)
    for r in res.results:
        acc += r["part"]
    out = acc + out_b[:, None] + x2
    return out.reshape(x.shape).astype(np.float32)
